# revision 2
# baseline (speedup 1.0000x reference)
"""Trainium2 Bass kernel v3 for mean Jaccard index (IoU) over 16 classes.

Structure per core (262144 px as (128, 2048)):
  Phase A (DMA-paced, ~48us): stream 16 fp32 pred planes; DVE runs the
    packed-max chain r = max(r, (bits(y_c)&~15)|c) (custom fused DVE op).
    Concurrently ACT runs the 15 ct cumulative sign passes on the int32
    target (ct depends only on target).
  Phase B (tail): idx = bits(r)&15 -> bf16; z = t + 16*(idx==t) (custom op,
    also accumulates sum(z) which yields ncorrect exactly given sum(t));
    31 remaining bins split: ACT 10 cumulative z passes (16.5..25.5),
    PE 14 is_equal masks (z 26..30 + cp 0..8), DVE 3 packed 2-bin custom
    accum passes (cp 9..14).
  Fold: per-partition accumulators summed by a ones matmul; host decodes
  exact fp64 counts -> IoU.
"""

import numpy as np

C = 16
B = 8
H = W = 512
PIX = H * W
P = 128
F = PIX // P  # 2048

CT_THRESH = list(range(15))          # ct cum thresholds c+0.5, c=0..14 (ACT, phase A)
ZA_THRESH = list(range(16, 26))      # z cum thresholds 16.5..25.5 (ACT, phase B)
PE_Z = [26, 27, 28, 29, 30]          # exact z bins via PE masks
PE_CP = list(range(9))               # cp bins 0..8 via PE masks
D2_CP = [(9, 10), (11, 12), (13, 14)]  # cp bin pairs via custom 2-bin DVE ops
N_PE = len(PE_Z) + len(PE_CP)
MASK_WAVE = 3

_cache = {}


def _register_custom_ops():
    import concourse.dve_ops as dve_ops
    from concourse.dve_ops import DveOp, OPS, _SUB_OPCODE_FOR_NAME, _CUSTOM_DVE_ROW_BASE
    from concourse.dve_spec import (
        Spec, Src0, Src1, C0, C1, C2, Zero, Bin, AluOp, select, eq, lower,
    )
    from concourse.dve_uop import DveOpSpec
    from operator import add

    existing = {o.name: o for o in OPS}
    if "PACKMAX_NOR" in existing and "BIN2_ACC" in existing:
        return existing

    def _mk(name, spec, subdim=False):
        shas = {}
        for ver in ("v3", "v4"):
            try:
                uops = lower(spec, ver=ver)
                shas[ver] = DveOpSpec(name=name, opcode=0, uops=uops, rd1_en=True).sha(ver)
            except Exception:
                pass
        op = DveOp(name, spec, subdim, shas)
        OPS.append(op)
        _SUB_OPCODE_FOR_NAME[op.name] = _CUSTOM_DVE_ROW_BASE + len(OPS) - 1
        dve_ops.CUSTOM_DVE_SPECS[op.name] = op.spec
        return op

    def _ref_packmax(in0, in1, s0, s1, imm2):
        y = in1.view(np.uint32)
        packed = (~((~y) | np.uint32(15))) | np.uint32(1)
        return np.maximum(in0, packed.view(np.float32))

    ny = Bin(AluOp.BITWISE_NOT, Src1, Src1)
    ory = Bin(AluOp.BITWISE_OR, ny, C0)
    masked = Bin(AluOp.BITWISE_NOT, ory, ory)
    packed = Bin(AluOp.BITWISE_OR, masked, C1)
    PACKMAX_NOR = _mk(
        "PACKMAX_NOR",
        Spec(body=Bin(AluOp.MAX, Src0, packed), reference=_ref_packmax),
    )

    # z = t + C0*(idx==t), accum_out = sum(z)
    Z_FUSED = _mk(
        "Z_FUSED",
        Spec(
            body=Src1 + select(eq(Src0, Src1), C0, Zero),
            accum=add,
            reference=lambda in0, in1, s0, s1, imm2: in1
            + s0 * (in0 == in1).astype(np.float32),
        ),
    )

    # accum_out = sum(eq(x,C0) + eq(x,C1)*C2); two bins field-packed (C2=4096)
    BIN2_ACC = _mk(
        "BIN2_ACC",
        Spec(
            body=eq(Src0, C0) + eq(Src0, C1) * C2,
            accum=add,
            reference=lambda in0, in1, s0, s1, imm2: (in0 == s0).astype(np.float32)
            + (in0 == s1).astype(np.float32) * imm2,
        ),
    )
    return {"PACKMAX_NOR": PACKMAX_NOR, "Z_FUSED": Z_FUSED, "BIN2_ACC": BIN2_ACC}


def _build_nc():
    import concourse.bacc as bacc
    import concourse.mybir as mybir
    import concourse.tile as tile

    ops = _register_custom_ops()
    Alu = mybir.AluOpType
    Act = mybir.ActivationFunctionType

    # accumulator columns: [ct 15][za 10][zsum 1][bin2 3] = 29 cols (P,29)
    NCT = len(CT_THRESH)
    NZA = len(ZA_THRESH)
    ND2 = len(D2_CP)
    NACC = NCT + NZA + 1 + ND2
    COL_ZSUM = NCT + NZA
    NOUT = NACC + N_PE

    nc = bacc.Bacc(target_bir_lowering=False, debug=False)
    pred = nc.dram_tensor("pred", [C, PIX], mybir.dt.float32, kind="ExternalInput")
    targ = nc.dram_tensor("target", [PIX], mybir.dt.int32, kind="ExternalInput")
    out = nc.dram_tensor("out", [1, NOUT], mybir.dt.float32, kind="ExternalOutput")
    out2 = nc.dram_tensor("out2", [P, ND2], mybir.dt.float32, kind="ExternalOutput")

    pred_r = pred[:].rearrange("c (p f) -> p c f", p=P)
    targ_r = targ[:].rearrange("(p f) -> p f", p=P)

    with tile.TileContext(nc) as tc:
        with (
            tc.tile_pool(name="planes", bufs=4) as planes,
            tc.tile_pool(name="big", bufs=1) as big,
            tc.tile_pool(name="mask", bufs=6) as maskp,
            tc.tile_pool(name="sc", bufs=1) as sc,
            tc.tile_pool(name="scr", bufs=2) as scr,
            tc.tile_pool(name="psum", bufs=2, space="PSUM") as psump,
            tc.tile_pool(name="psum2", bufs=1, space="PSUM") as psump2,
        ):
            # ---- first DMAs queued before anything else: plane 0, 1, target
            y0 = planes.tile([P, F], mybir.dt.float32, tag="y")
            nc.sync.dma_start(out=y0[:], in_=pred_r[:, 0, :])
            ti = big.tile([P, F], mybir.dt.int32)
            nc.sync.dma_start(out=ti[:], in_=targ_r[:, :])

            # ---- constants
            consts = sc.tile([P, 18], mybir.dt.uint32)
            nc.vector.memset(consts[:, 0:1], 15)
            for c in range(1, C):
                nc.vector.memset(consts[:, c : c + 1], c)
            c15 = consts[:, 0:1].bitcast(mybir.dt.float32)
            classc = [consts[:, c : c + 1].bitcast(mybir.dt.float32) for c in range(1, C)]

            biasct = sc.tile([P, NCT], mybir.dt.float32)
            for j, c in enumerate(CT_THRESH):
                nc.vector.memset(biasct[:, j : j + 1], -(c + 0.5))
            biasza = sc.tile([P, NZA], mybir.dt.float32)
            for j, v in enumerate(ZA_THRESH):
                nc.vector.memset(biasza[:, j : j + 1], -(v + 0.5))

            ones = sc.tile([P, 1], mybir.dt.bfloat16)
            nc.vector.memset(ones[:], 1.0)
            onesf = sc.tile([P, 1], mybir.dt.float32)
            nc.vector.memset(onesf[:], 1.0)

            accum = sc.tile([P, NACC], mybir.dt.float32)

            # ---- packed-max chain (DVE), fed plane by plane
            r = big.tile([P, F], mybir.dt.float32)
            nc.vector.tensor_scalar(
                r[:].bitcast(mybir.dt.uint32),
                y0[:].bitcast(mybir.dt.uint32),
                0xFFFFFFF0,
                0,
                Alu.bitwise_and,
                Alu.bitwise_or,
            )
            for c in range(1, C):
                yc = planes.tile([P, F], mybir.dt.float32, tag="y")
                nc.sync.dma_start(out=yc[:], in_=pred_r[:, c, :])
                nc.vector._custom_dve(
                    ops["PACKMAX_NOR"], out=r[:], in0=r[:], in1=yc[:],
                    s0=c15, s1=classc[c - 1],
                )

            # ---- phase A on ACT: t cast + ct cumulative passes (int32 input)
            tb = big.tile([P, F], mybir.dt.bfloat16)
            nc.scalar.activation(tb[:], ti[:], Act.Copy, scale=1.0)
            for j in range(NCT):
                sa = scr.tile([P, F], mybir.dt.bfloat16, tag="sa")
                nc.scalar.activation(
                    sa[:], ti[:], Act.Sign,
                    bias=biasct[:, j : j + 1], scale=1.0,
                    accum_out=accum[:, j : j + 1],
                )

            # ---- phase B prep: idx, z
            idxu = big.tile([P, F], mybir.dt.uint32)
            nc.vector.tensor_scalar(
                idxu[:], r[:].bitcast(mybir.dt.uint32), 15, None, Alu.bitwise_and
            )
            idxb = big.tile([P, F], mybir.dt.bfloat16)
            nc.vector.tensor_copy(idxb[:], idxu[:])
            zb = big.tile([P, F], mybir.dt.bfloat16)
            nc.vector._custom_dve(
                ops["Z_FUSED"], out=zb[:], in0=idxb[:], in1=tb[:], s0=16.0,
                accum_out=accum[:, COL_ZSUM : COL_ZSUM + 1],
            )

            # ---- phase B bins
            # ACT: z cumulative passes
            for j in range(NZA):
                sa = scr.tile([P, F], mybir.dt.bfloat16, tag="sa")
                nc.scalar.activation(
                    sa[:], zb[:], Act.Sign,
                    bias=biasza[:, j : j + 1], scale=1.0,
                    accum_out=accum[:, NCT + j : NCT + j + 1],
                )
            # DVE: packed 2-bin custom passes on idx
            for j, (a, b) in enumerate(D2_CP):
                col = NCT + NZA + 1 + j
                d2o = scr.tile([P, F], mybir.dt.bfloat16, tag="sa")
                nc.vector._custom_dve(
                    ops["BIN2_ACC"], out=d2o[:], in0=idxb[:],
                    s0=float(a), s1=float(b), imm2=4096.0,
                    accum_out=accum[:, col : col + 1],
                )
            # PE: is_equal masks summed via matmuls
            pe_counts = sc.tile([1, N_PE], mybir.dt.float32)
            specs = [(zb, float(v)) for v in PE_Z] + [(idxb, float(v)) for v in PE_CP]
            nwave = (N_PE + MASK_WAVE - 1) // MASK_WAVE
            for w in range(nwave):
                lo = w * MASK_WAVE
                hi = min(lo + MASK_WAVE, N_PE)
                ps = psump.tile([1, MASK_WAVE, 512], mybir.dt.float32, tag="ps")
                for b in range(lo, hi):
                    src, val = specs[b]
                    mk = maskp.tile([P, F], mybir.dt.bfloat16, tag="mk")
                    nc.vector.tensor_scalar(mk[:], src[:], val, None, Alu.is_equal)
                    mv = mk[:].rearrange("p (s f) -> p s f", s=F // 512)
                    for k in range(F // 512):
                        nc.tensor.matmul(
                            ps[:, b - lo, :], ones[:], mv[:, k, :],
                            start=(k == 0), stop=(k == F // 512 - 1),
                        )
                nc.vector.tensor_reduce(
                    out=pe_counts[:, lo:hi],
                    in_=ps[:, : hi - lo, :],
                    axis=mybir.AxisListType.X,
                    op=Alu.add,
                )

            # ---- fold accumulators over partitions; emit
            NF = NCT + NZA + 1
            ps2 = psump2.tile([1, NF], mybir.dt.float32, tag="ps2")
            nc.tensor.matmul(ps2[:], onesf[:], accum[:, :NF], start=True, stop=True)
            outsb = sc.tile([1, NOUT], mybir.dt.float32)
            nc.vector.memset(outsb[:, NF:NACC], 0.0)
            nc.scalar.copy(outsb[:, :NF], ps2[:])
            nc.vector.tensor_copy(outsb[:, NACC:], pe_counts[:])
            nc.sync.dma_start(out=out[:], in_=outsb[:])
            nc.sync.dma_start(out=out2[:], in_=accum[:, NF:NACC])

    nc.finalize()
    return nc


def _get_nc():
    if "nc" not in _cache:
        _cache["nc"] = _build_nc()
    return _cache["nc"]


def _decode(outs):
    NCT = len(CT_THRESH)
    NZA = len(ZA_THRESH)
    n_cores = len(outs)
    ntot = float(PIX) * n_cores

    tot = np.zeros(np.asarray(outs[0][0]).size, dtype=np.float64)
    bin2 = np.zeros((P, len(D2_CP)), dtype=np.float64)
    for o, o2 in outs:
        tot += np.asarray(o, dtype=np.float64).reshape(-1)
        bin2 += np.asarray(o2, dtype=np.float64)

    NACC = NCT + NZA + 1 + len(D2_CP)
    ct_T = tot[:NCT]
    za_T = tot[NCT : NCT + NZA]
    zsum = tot[NCT + NZA]
    pe = tot[NACC:]

    # ct histogram from cumulative counts
    ct_cum = (ntot - ct_T) / 2.0          # #(t <= c), c = 0..14
    ct = np.zeros(C)
    ct[0] = ct_cum[0]
    for c in range(1, 15):
        ct[c] = ct_cum[c] - ct_cum[c - 1]
    ct[15] = ntot - ct_cum[14]

    # ncorrect from sum(z) = sum(t) + 16*ncorrect
    sum_t = float(sum(c * ct[c] for c in range(C)))
    ncorrect = (zsum - sum_t) / 16.0

    # z histogram bins 16..31
    zh = np.zeros(32)
    za_cum = (ntot - za_T) / 2.0          # #(z <= v), v = 16..25
    cum15 = ntot - ncorrect               # #(z <= 15)
    zh[16] = za_cum[0] - cum15
    for j in range(1, NZA):
        zh[16 + j] = za_cum[j] - za_cum[j - 1]
    for i, v in enumerate(PE_Z):
        zh[v] = pe[i]
    zh[31] = ncorrect - zh[16:31].sum()

    inter = zh[16:32].copy()

    # cp bins
    cp = np.zeros(C)
    for i, c in enumerate(PE_CP):
        cp[c] = pe[len(PE_Z) + i]
    for j, (a, b) in enumerate(D2_CP):
        v = bin2[:, j]                      # per-partition packed, exact
        nb = np.floor(v / 4096.0)
        na = v - 4096.0 * nb
        cp[a] = na.sum()
        cp[b] = nb.sum()
    cp[15] = ntot - cp[:15].sum()

    union = cp + ct - inter
    scores = np.where(union == 0, 1.0, inter / np.where(union == 0, 1.0, union))
    return scores.mean()


def run(pred, target, trace=False):
    from concourse.bass_utils import run_bass_kernel_spmd

    pred = np.asarray(pred, dtype=np.float32)
    target = np.asarray(target, dtype=np.int32)
    assert pred.shape == (B, C, H, W), pred.shape
    assert target.shape == (B, H, W), target.shape

    nc = _get_nc()
    in_maps = [
        {
            "pred": np.ascontiguousarray(pred[b]).reshape(C, PIX),
            "target": np.ascontiguousarray(target[b]).reshape(PIX),
        }
        for b in range(B)
    ]
    res = run_bass_kernel_spmd(nc, in_maps, core_ids=list(range(B)), trace=trace)
    outs = [(r["out"], r["out2"]) for r in res.results]
    return np.float32(_decode(outs)), res


def kernel(pred, target):
    result, _ = run(pred, target)
    return np.asarray(result, dtype=np.float32)
